# revision 50
# baseline (speedup 1.0000x reference)
"""RNN-T Joint network kernel for 8x Trainium2 NeuronCores.

logits[b,t,u,v] = enc_out[b,t,:] @ W[v,:512] + pred_out[b,u,:] @ W[v,512:] + b[v]

Sharding: data-parallel over (B=4) x (T split in 2) -> 8 shards.
Core i handles b = i//2, t in [128*(i%2), 128*(i%2)+128).
Each core computes a (128, 64, 2048) output slab, stored as bf16
(32 MiB; rounding ~2e-3 relative, well inside the 2e-2 gate) and
upcast to f32 on the host after gather.

v3 design (vs the v1 streaming-transpose kernel, 212 us): all DMA
queues feed the same 16 SDMA engines and share the per-core HBM
budget (~245 GB/s measured for sustained stores), so the floor is set
by bytes moved, not ring count. v1 moved 32 MiB (stores) + 8 MiB
(W reload) every pass and burned PE/DVE/ScalarE on transposing W on
the fly. v3 (125 us measured):
  - W^T, enc^T, pred^T are pre-transposed on the HOST and fed in
    k-tile-major layout, so the device does no transposes at all.
  - W^T (8 MiB, f32r) is loaded once into SBUF and stays RESIDENT;
    the steady-state pass moves only the 32 MiB of output stores.
  - f32r for the projection matmuls (W stays full fp32 bits), bf16
    for the broadcast path (pred_c/enc_c/onehot/ident + bias): bf16
    moving operands run ~2x faster on HW than f32r ones.
  - 2 MiB output stores (16 u's per store) amortize per-DMA overhead.

Per v-chunk (512 cols): enc/pred projection chains on two PSUM banks,
bias folded into the pred chain via a ones-row matmul; ScalarE copies
the small projections to SBUF as f32r. Main loop per PSUM tile (2 u's):
broadcast pred_proj row u across the 128 t-partitions with a
dense-onehot f32r matmul. Tiles alternate between two drain paths to
balance engines: DVE tensor_add (enc broadcast via stride-0 free-dim
AP) -> bf16 out tile, or PE accumulating enc via identity-matmul with
ScalarE doing the PSUM->SBUF bf16 copy.

Config search (all measured with the paired-median harness below;
medians of the per-config pair clusters):
  - act2/sync/gu16 (this config): 111-134 us across runs.
  - More DVE drain share (act_mod=4/8): 250 us. More ACT: untested
    worse direction. The half/half act_mod=2 split wins clearly.
  - Stores on 2-3 DMA queues (alt/rot3/rot2g): 263-303 us. Stores
    MUST stay on nc.sync only.
  - 4 MiB stores (gu32): 166 us. K=128 broadcast (0.5-duplicated
    onehot): no PE change. Materialized [enc|enc] operand instead of
    the stride-0 broadcast AP in the DVE add (enc_wide): 132 us, no
    change. f32r broadcast path (v2): ~218-250 us.
  - int8 output (host folds 1/s into W,b; s from the exact separable
    max; host dequant): 113 us, rel err 6.7e-3 -- kept OFF (OUT_I8):
    within run-to-run noise of bf16, not worth gate-margin risk.
Timing methodology (bench.py + test.py): reps=24/48 kernels
(exec-bound chains, donated outputs), interleaved lo/hi pairs, median
of pair diffs. Beware: compute-only microbench kernels (no stores)
get dead-code-eliminated -- only variants that write out_d measure
real work. A recurring ~+40 ms per-chain host contamination produces
occasional near-zero pairs; the median rejects them.
"""

import numpy as np

B, T, U = 4, 256, 64
D_ENC, D_PRED, VOCAB = 512, 512, 2048
D = D_ENC + D_PRED
TT = 128  # t rows per core
N_CORES = 8
NC = 4          # v chunks
VC = VOCAB // NC  # 512 v per chunk
KD = D // 128   # 8 k-tiles along d
KE = D_ENC // 128  # 4 enc k-tiles

# bump on ANY functional change to the BIR body: the neuron compile
# cache keys on HLO-level shapes only (the BIR is smuggled through a
# custom_call), so two builds with identical tensor shapes collide.
BUILD_ID = 4

_cache = {}


def _build(reps=1, gu=16, act_mod=2, store_mode="sync", w_f32r=True,
           first_gp=False, mode="full", out_i8=False, enc_wide=False,
           obufs=3):
    """gu: u's per store group. act_mod: every act_mod-th PSUM tile takes
    the ScalarE drain path (2 -> half). store_mode: which DMA queues the
    output stores rotate over -- 'sync' (one HWDGE ring), 'alt' (both
    HWDGE rings), 'rot3' (both HWDGE + the SWDGE/gpsimd queue). Each
    queue serializes transfer + ~2us completion receipt per DMA, so
    spreading stores over queues overlaps those gaps."""
    import concourse.bacc as bacc
    import concourse.mybir as mybir
    from concourse.tile import TileContext

    f32 = mybir.dt.float32
    f32r = mybir.dt.float32r
    bf16 = mybir.dt.bfloat16

    nc = bacc.Bacc("TRN2", target_bir_lowering=False, debug=False, num_devices=N_CORES)
    # config marker: shape encodes build flags so the compile cache
    # (which doesn't hash the BIR) can't serve a stale NEFF
    nc.dram_tensor("cfg",
                   (1 + BUILD_ID * 64 + gu + 2 * act_mod
                    + 512 * ["sync", "alt", "rot3", "rot2g"].index(store_mode)
                    + 2048 * int(w_f32r)
                    + 4096 * int(first_gp)
                    + 8192 * ["full", "pe_only", "nostore",
                              "store_only"].index(mode)
                    + 32768 * int(out_i8)
                    + 65536 * int(enc_wide)
                    + 262144 * (obufs - 3), reps),
                   f32, kind="ExternalInput")
    wdt = f32r if w_f32r else bf16
    # host-pretransposed operands, k-tile-major with partition outermost
    wt_d = nc.dram_tensor("wt", (128, KD, VOCAB), wdt, kind="ExternalInput")
    enc_t_d = nc.dram_tensor("enc_t", (128, KE, TT), wdt, kind="ExternalInput")
    pred_t_d = nc.dram_tensor("pred_t", (128, KE, U), wdt, kind="ExternalInput")
    # bias | 1.0 packed on one partition; bf16 so the bias matmul runs
    # at native rate (b is ~0.01-scale, bf16 rounding is negligible)
    const_d = nc.dram_tensor("consts", (1, VOCAB + 1), bf16, kind="ExternalInput")
    # bf16 broadcast path: onehot/ident selection matrices are exact in
    # bf16, and the PE runs 1 cyc/col for bf16 moving operands (f32r
    # measured ~2x slower on HW despite the cost model saying 1x).
    # (A K=128 variant with 0.5-duplicated onehot rows and pred mirrored
    # across partition halves measured the same PE rate and a slower
    # full kernel -- reverted.)
    onehot_d = nc.dram_tensor("onehotr", (U, U * 128), bf16, kind="ExternalInput")
    identr_d = nc.dram_tensor("identr", (128, 128), bf16, kind="ExternalInput")
    odt = mybir.dt.int8 if out_i8 else bf16
    out_d = nc.dram_tensor("out", (TT, U, VOCAB), odt, kind="ExternalOutput")

    GU = gu
    n_groups = U // GU

    with TileContext(nc) as tc:
        with (
            tc.tile_pool(name="const", bufs=1) as const,
            tc.tile_pool(name="persist", bufs=1) as persist,
            tc.tile_pool(name="projc", bufs=2) as projc,
            tc.tile_pool(name="outp", bufs=obufs if gu <= 16 else 2) as outp,
            tc.tile_pool(name="ps_s", bufs=2, space="PSUM") as ps_s,
            tc.tile_pool(name="ps_m", bufs=3, space="PSUM") as ps_m,
        ):
            consts = const.tile([1, VOCAB + 1], bf16)
            bias_sb = consts[0:1, 0:VOCAB]
            ones = consts[0:1, VOCAB:VOCAB + 1]
            # bf16 identity: column u broadcast over 128 cols is the lhsT
            # that broadcasts pred_proj row u across 128 output partitions;
            # identr is the lhsT that accumulates enc_proj into PSUM on
            # the ScalarE-copy path.
            onehot = const.tile([U, U * 128], bf16)
            identr = const.tile([128, 128], bf16)

            w_sb = persist.tile([128, KD * VOCAB], wdt)
            w3 = w_sb.rearrange("p (k v) -> p k v", k=KD)
            enc_t = persist.tile([128, KE * TT], wdt)
            pred_t = persist.tile([128, KE * U], wdt)

            def _prologue():
                # small operands lead on the scalar ring; onehot/identr on
                # sync. W^T chunk-0 slab split across both HWDGE rings so
                # chunk 0's projections can start ASAP; chunks 1-3 slabs
                # prefetch on the gpsimd queue, overlapping chunk 0.
                nc.scalar.dma_start(out=enc_t, in_=enc_t_d[:])
                nc.scalar.dma_start(out=pred_t, in_=pred_t_d[:])
                nc.scalar.dma_start(out=consts, in_=const_d[:])
                nc.sync.dma_start(out=identr, in_=identr_d[:])
                nc.sync.dma_start(out=onehot, in_=onehot_d[:])
                nc.scalar.dma_start(out=w3[:, 0:KE, 0:VC],
                                    in_=wt_d[:, 0:KE, 0:VC])
                nc.sync.dma_start(out=w3[:, KE:KD, 0:VC],
                                  in_=wt_d[:, KE:KD, 0:VC])
                eng = nc.gpsimd if first_gp else nc.sync
                for c in range(1, NC):
                    eng.dma_start(out=w3[:, :, c * VC:(c + 1) * VC],
                                  in_=wt_d[:, :, c * VC:(c + 1) * VC])
                # PE pre-consumes the onehot/identr DMAs once so no later
                # instruction needs >2 sync-wait commands (ISA limit)
                ps_dummy = ps_s.tile([128, VC], f32, tag="ps")
                nc.tensor.matmul(ps_dummy[:, :128], lhsT=identr, rhs=identr,
                                 start=True, stop=True)
                nc.tensor.matmul(ps_dummy, lhsT=onehot[:, 0:128],
                                 rhs=onehot[:, 0:VC], start=True, stop=True)

            def _chunk(c, first=False):
                if first:
                    _prologue()
                # -- enc/pred projection chunks; the two accumulation chains
                # interleave across two PSUM banks so each matmul's drain
                # overlaps the other chain's fill
                ps = ps_s.tile([128, VC], f32, tag="ps")
                ps2 = ps_s.tile([128, VC], f32, tag="ps")
                for k in range(KE):
                    nc.tensor.matmul(
                        ps, lhsT=enc_t[:, k * TT:(k + 1) * TT],
                        rhs=w3[:, k, c * VC:(c + 1) * VC],
                        start=(k == 0), stop=(k == KE - 1))
                    nc.tensor.matmul(
                        ps2[:U], lhsT=pred_t[:, k * U:(k + 1) * U],
                        rhs=w3[:, KE + k, c * VC:(c + 1) * VC],
                        start=(k == 0), stop=False)
                if enc_wide:
                    # enc duplicated side by side: the DVE drain then adds
                    # with unit strides instead of a stride-0 broadcast AP
                    enc2 = projc.tile([128, 2 * VC], bf16, tag="enc_c")
                    enc_c = enc2[:, 0:VC]
                    nc.scalar.copy(out=enc_c, in_=ps)
                    nc.scalar.copy(out=enc2[:, VC:2 * VC], in_=ps)
                else:
                    enc2 = None
                    enc_c = projc.tile([128, VC], bf16, tag="enc_c")
                    nc.scalar.copy(out=enc_c, in_=ps)
                nc.tensor.matmul(
                    ps2[:U], lhsT=ones.broadcast_to((1, U)),
                    rhs=bias_sb[:, c * VC:(c + 1) * VC],
                    start=False, stop=True)
                pred_c = projc.tile([U, VC], bf16, tag="pred_c")
                nc.scalar.copy(out=pred_c, in_=ps2[:U])

                # -- main: stores of (128t, GU u, 512v) bf16. Each PSUM
                # tile covers 2 u's; every act_mod-th tile takes the
                # ScalarE path (PE also accumulates enc via identity-
                # matmul, ScalarE copies PSUM->SBUF), the rest the DVE
                # path (tensor_add with free-dim-broadcast enc operand).
                for g in range(n_groups):
                    o = outp.tile([128, GU * VC], odt, tag="o")
                    if mode == "store_only":
                        nc.vector.tensor_copy(o[0:1, 0:1], ones)
                    for h in range(GU // 2):
                        if mode == "store_only":
                            break
                        use_act = (h % act_mod == 0)
                        ps = ps_m.tile([128, 2 * VC], f32, tag="ps")
                        for s in range(2):
                            u = g * GU + h * 2 + s
                            nc.tensor.matmul(
                                ps[:, s * VC:(s + 1) * VC],
                                lhsT=onehot[:, u * 128:(u + 1) * 128],
                                rhs=pred_c, start=True, stop=not use_act)
                        if use_act:
                            for s in range(2):
                                nc.tensor.matmul(
                                    ps[:, s * VC:(s + 1) * VC],
                                    lhsT=identr, rhs=enc_c,
                                    start=False, stop=True)
                        if mode == "pe_only":
                            continue
                        dst = o[:, h * 2 * VC:(h + 1) * 2 * VC]
                        if use_act:
                            nc.scalar.copy(out=dst, in_=ps)
                        elif enc_wide:
                            nc.vector.tensor_add(dst, enc2, ps)
                        else:
                            nc.vector.tensor_add(
                                dst.rearrange("p (a v) -> p a v", a=2),
                                enc_c[:, None, :].broadcast_to((128, 2, VC)),
                                ps.rearrange("p (a v) -> p a v", a=2))
                    if mode in ("pe_only", "nostore"):
                        continue
                    gi = c * n_groups + g
                    if store_mode == "alt":
                        seng = (nc.sync, nc.scalar)[gi % 2]
                    elif store_mode == "rot3":
                        seng = (nc.sync, nc.scalar, nc.gpsimd)[gi % 3]
                    elif store_mode == "rot2g":
                        seng = (nc.sync, nc.gpsimd)[gi % 2]
                    else:
                        seng = nc.sync
                    seng.dma_start(
                        out=out_d[:, g * GU:(g + 1) * GU, c * VC:(c + 1) * VC],
                        in_=o.rearrange("p (a v) -> p a v", a=GU))

            for _rep in range(reps):
                for c in range(NC):
                    _chunk(c, first=(_rep == 0 and c == 0))
    nc.compile()
    return nc


def _i8_scale(enc_out, pred_out, W, b):
    """Exact global max of |logits|: max/min over (t,u) separate, so
    max_{t,u}(e+p) = max_t e + max_u p exactly, per (batch, v)."""
    enc = np.asarray(enc_out, dtype=np.float32)
    pred = np.asarray(pred_out, dtype=np.float32)
    W = np.asarray(W, dtype=np.float32)
    b = np.asarray(b, dtype=np.float32)
    d = enc.shape[-1]
    e = enc.reshape(-1, d) @ W[:, :d].T          # (B*T, V)
    p = pred.reshape(-1, d) @ W[:, d:].T         # (B*U, V)
    e = e.reshape(B, T, VOCAB)
    p = p.reshape(B, U, VOCAB)
    hi = (e.max(1) + p.max(1) + b).max()
    lo = (e.min(1) + p.min(1) + b).min()
    return float(max(hi, -lo)) / 126.0


def _make_in_maps(enc_out, pred_out, W, b, reps=1, gu=16, act_mod=2,
                  store_mode="sync", w_f32r=True, first_gp=False,
                  mode="full", out_i8=False, enc_wide=False, obufs=3):
    import ml_dtypes
    bf16 = ml_dtypes.bfloat16
    wnp = np.float32 if w_f32r else bf16
    W = np.asarray(W, dtype=np.float32)
    if out_i8:
        s = _i8_scale(enc_out, pred_out, W, b)
        W = W / s
        b = np.asarray(b, dtype=np.float32) / s
    # W^T in k-tile-major (128 d-partition, KD k, V) layout
    wt = np.ascontiguousarray(
        W.T.reshape(KD, 128, VOCAB).transpose(1, 0, 2)).astype(wnp)
    consts = np.zeros((1, VOCAB + 1), dtype=np.float32)
    consts[0, :VOCAB] = np.asarray(b, dtype=np.float32)
    consts[0, VOCAB] = 1.0
    consts = consts.astype(bf16)
    ident = np.eye(128, dtype=np.float32).astype(bf16)
    onehot_m = np.zeros((U, U * 128), dtype=np.float32)
    for u in range(U):
        onehot_m[u, u * 128:(u + 1) * 128] = 1.0
    onehot_m = onehot_m.astype(bf16)
    cfg = np.zeros((1 + BUILD_ID * 64 + gu + 2 * act_mod
                    + 512 * ["sync", "alt", "rot3", "rot2g"].index(store_mode)
                    + 2048 * int(w_f32r)
                    + 4096 * int(first_gp)
                    + 8192 * ["full", "pe_only", "nostore",
                              "store_only"].index(mode)
                    + 32768 * int(out_i8)
                    + 65536 * int(enc_wide)
                    + 262144 * (obufs - 3), reps),
                   dtype=np.float32)
    in_maps = []
    for i in range(N_CORES):
        bi, th = i // 2, i % 2
        enc = np.asarray(enc_out[bi, th * TT:(th + 1) * TT, :],
                         dtype=np.float32)  # (TT, D_ENC)
        pred = np.asarray(pred_out[bi], dtype=np.float32)  # (U, D_PRED)
        enc_t = np.ascontiguousarray(
            enc.T.reshape(KE, 128, TT).transpose(1, 0, 2)).astype(wnp)
        pred_t = np.ascontiguousarray(
            pred.T.reshape(KE, 128, U).transpose(1, 0, 2)).astype(wnp)
        in_maps.append({
            "cfg": cfg,
            "wt": wt,
            "enc_t": enc_t,
            "pred_t": pred_t,
            "consts": consts,
            "identr": ident,
            "onehotr": onehot_m,
        })
    return in_maps


OUT_I8 = False


def kernel(enc_out, pred_out, W, b):
    import os

    from concourse.bass_utils import run_bass_kernel_spmd

    if "nc" not in _cache:
        _cache["nc"] = _build(out_i8=OUT_I8)
    nc = _cache["nc"]
    trace = bool(os.environ.get("KJN_TRACE"))

    in_maps = _make_in_maps(enc_out, pred_out, W, b, out_i8=OUT_I8)
    s = _i8_scale(enc_out, pred_out, W, b) if OUT_I8 else 1.0

    kw = {}
    if trace:
        kw = dict(trace=True, trace_cores=[0], stitch_traces=False)
    res = run_bass_kernel_spmd(nc, in_maps, core_ids=list(range(N_CORES)), **kw)
    if trace:
        print(f"HW exec time: {res.exec_time_ns} ns")
        print(f"trace: {res.instructions_and_trace[1] if res.instructions_and_trace else None}")
        print(f"profile_json: {res.profile_json}")
    out = np.empty((B, T, U, VOCAB), dtype=np.float32)
    for i in range(N_CORES):
        bi, th = i // 2, i % 2
        slab = np.asarray(res.results[i]["out"], dtype=np.float32)
        if OUT_I8:
            slab *= s
        out[bi, th * TT:(th + 1) * TT] = slab
    return out


# revision 61
# speedup vs baseline: 1.1179x; 1.1179x over previous
"""RNN-T Joint network kernel for 8x Trainium2 NeuronCores.

logits[b,t,u,v] = enc_out[b,t,:] @ W[v,:512] + pred_out[b,u,:] @ W[v,512:] + b[v]

Sharding: data-parallel over (B=4) x (T split in 2) -> 8 shards.
Core i handles b = i//2, t in [128*(i%2), 128*(i%2)+128).
Each core computes a (128, 64, 2048) output slab, stored as bf16
(32 MiB; rounding ~2e-3 relative, well inside the 2e-2 gate) and
upcast to f32 on the host after gather.

v3 design (vs the v1 streaming-transpose kernel, 212 us): all DMA
queues feed the same 16 SDMA engines and share the per-core HBM
budget (~245 GB/s measured for sustained stores), so the floor is set
by bytes moved, not ring count. v1 moved 32 MiB (stores) + 8 MiB
(W reload) every pass and burned PE/DVE/ScalarE on transposing W on
the fly. v3 (125 us measured):
  - W^T, enc^T, pred^T are pre-transposed on the HOST and fed in
    k-tile-major layout, so the device does no transposes at all.
  - W^T (8 MiB, f32r) is loaded once into SBUF and stays RESIDENT;
    the steady-state pass moves only the 32 MiB of output stores.
  - f32r for the projection matmuls (W stays full fp32 bits), bf16
    for the broadcast path (pred_c/enc_c/onehot/ident + bias): bf16
    moving operands run ~2x faster on HW than f32r ones.
  - 2 MiB output stores (16 u's per store) amortize per-DMA overhead.

Per v-chunk (512 cols): enc/pred projection chains on two PSUM banks,
bias folded into the pred chain via a ones-row matmul; ScalarE copies
the small projections to SBUF as f32r. Main loop per PSUM tile (2 u's):
broadcast pred_proj row u across the 128 t-partitions with a
dense-onehot f32r matmul. Tiles alternate between two drain paths to
balance engines: DVE tensor_add (enc broadcast via stride-0 free-dim
AP) -> bf16 out tile, or PE accumulating enc via identity-matmul with
ScalarE doing the PSUM->SBUF bf16 copy.

Config search (all measured with the paired-median harness below;
medians of the per-config pair clusters):
  - act2/sync/gu16 (this config): 111-134 us across runs.
  - More DVE drain share (act_mod=4/8): 250 us. More ACT: untested
    worse direction. The half/half act_mod=2 split wins clearly.
  - Stores on 2-3 DMA queues (alt/rot3/rot2g): 263-303 us. Stores
    MUST stay on nc.sync only.
  - 4 MiB stores (gu32): 166 us. K=128 broadcast (0.5-duplicated
    onehot): no PE change. Materialized [enc|enc] operand instead of
    the stride-0 broadcast AP in the DVE add (enc_wide): 132 us, no
    change. f32r broadcast path (v2): ~218-250 us.
  - int8 output (host folds 1/s into W,b; s from the exact separable
    max; host dequant): 113 us, rel err 6.7e-3 -- kept OFF (OUT_I8):
    within run-to-run noise of bf16, not worth gate-margin risk.
  - 1 MiB stores (gu8): 128 us, within noise of gu16. Finer store
    slices per group (store_split): untested beyond gu8 equivalence.
  - v-partitioned mode (mode='vpart': v on partitions, projections
    via W^T-as-lhsT, both broadcasts as free-dim stride-0 APs, adds
    split DVE+GPSIMD, no PE broadcast, (v,t,u) stores + host
    transpose): correct at 4.7e-3 but 180 us -- the DVE/GPSIMD
    double-broadcast adds are slower than the PE-broadcast pipeline.
Timing methodology (bench.py + test.py): reps=24/48 kernels
(exec-bound chains, donated outputs), interleaved lo/hi pairs, median
of pair diffs. Beware: compute-only microbench kernels (no stores)
get dead-code-eliminated -- only variants that write out_d measure
real work. A recurring ~+40 ms per-chain host contamination produces
occasional near-zero pairs; the median rejects them.
"""

import numpy as np

B, T, U = 4, 256, 64
D_ENC, D_PRED, VOCAB = 512, 512, 2048
D = D_ENC + D_PRED
TT = 128  # t rows per core
N_CORES = 8
NC = 4          # v chunks
VC = VOCAB // NC  # 512 v per chunk
KD = D // 128   # 8 k-tiles along d
KE = D_ENC // 128  # 4 enc k-tiles

# bump on ANY functional change to the BIR body: the neuron compile
# cache keys on HLO-level shapes only (the BIR is smuggled through a
# custom_call), so two builds with identical tensor shapes collide.
BUILD_ID = 5

_cache = {}


def _build(reps=1, gu=16, act_mod=2, store_mode="sync", w_f32r=True,
           first_gp=False, mode="full", out_i8=False, enc_wide=False,
           obufs=3, store_split=1):
    """gu: u's per store group. act_mod: every act_mod-th PSUM tile takes
    the ScalarE drain path (2 -> half). store_mode: which DMA queues the
    output stores rotate over -- 'sync' (one HWDGE ring), 'alt' (both
    HWDGE rings), 'rot3' (both HWDGE + the SWDGE/gpsimd queue). Each
    queue serializes transfer + ~2us completion receipt per DMA, so
    spreading stores over queues overlaps those gaps."""
    import concourse.bacc as bacc
    import concourse.mybir as mybir
    from concourse.tile import TileContext

    f32 = mybir.dt.float32
    f32r = mybir.dt.float32r
    bf16 = mybir.dt.bfloat16

    nc = bacc.Bacc("TRN2", target_bir_lowering=False, debug=False, num_devices=N_CORES)
    # config marker: shape encodes build flags so the compile cache
    # (which doesn't hash the BIR) can't serve a stale NEFF
    nc.dram_tensor("cfg",
                   (1 + BUILD_ID * 64 + gu + 2 * act_mod
                    + 512 * ["sync", "alt", "rot3", "rot2g"].index(store_mode)
                    + 2048 * int(w_f32r)
                    + 4096 * int(first_gp)
                    + 8192 * ["full", "pe_only", "nostore",
                              "store_only", "vpart"].index(mode)
                    + 32768 * int(out_i8)
                    + 65536 * int(enc_wide)
                    + 262144 * (obufs - 3)
                    + 524288 * (store_split - 1), reps),
                   f32, kind="ExternalInput")
    wdt = f32r if w_f32r else bf16
    # host-pretransposed operands, k-tile-major with partition outermost
    wt_d = nc.dram_tensor("wt", (128, KD, VOCAB), wdt, kind="ExternalInput")
    enc_t_d = nc.dram_tensor("enc_t", (128, KE, TT), wdt, kind="ExternalInput")
    pred_t_d = nc.dram_tensor("pred_t", (128, KE, U), wdt, kind="ExternalInput")
    # bias | 1.0 packed on one partition; bf16 so the bias matmul runs
    # at native rate (b is ~0.01-scale, bf16 rounding is negligible)
    const_d = nc.dram_tensor("consts", (1, VOCAB + 1), bf16, kind="ExternalInput")
    # bf16 broadcast path: onehot/ident selection matrices are exact in
    # bf16, and the PE runs 1 cyc/col for bf16 moving operands (f32r
    # measured ~2x slower on HW despite the cost model saying 1x).
    # (A K=128 variant with 0.5-duplicated onehot rows and pred mirrored
    # across partition halves measured the same PE rate and a slower
    # full kernel -- reverted.)
    onehot_d = nc.dram_tensor("onehotr", (U, U * 128), bf16, kind="ExternalInput")
    identr_d = nc.dram_tensor("identr", (128, 128), bf16, kind="ExternalInput")
    odt = mybir.dt.int8 if out_i8 else bf16
    if mode == "vpart":
        # v on partitions: output in (v, t, u) layout, host transposes
        out_d = nc.dram_tensor("out", (VOCAB, TT, U), odt,
                               kind="ExternalOutput")
    else:
        out_d = nc.dram_tensor("out", (TT, U, VOCAB), odt,
                               kind="ExternalOutput")

    GU = gu
    n_groups = U // GU

    with TileContext(nc) as tc:
        with (
            tc.tile_pool(name="const", bufs=1) as const,
            tc.tile_pool(name="persist", bufs=1) as persist,
            tc.tile_pool(name="projc", bufs=2) as projc,
            tc.tile_pool(name="outp", bufs=obufs if gu <= 16 else 2) as outp,
            tc.tile_pool(name="ps_s", bufs=2, space="PSUM") as ps_s,
            tc.tile_pool(name="ps_m", bufs=3, space="PSUM") as ps_m,
        ):
            consts = const.tile([1, VOCAB + 1], bf16)
            bias_sb = consts[0:1, 0:VOCAB]
            ones = consts[0:1, VOCAB:VOCAB + 1]
            # bf16 identity: column u broadcast over 128 cols is the lhsT
            # that broadcasts pred_proj row u across 128 output partitions;
            # identr is the lhsT that accumulates enc_proj into PSUM on
            # the ScalarE-copy path.
            onehot = const.tile([U, U * 128], bf16)
            identr = const.tile([128, 128], bf16)

            w_sb = persist.tile([128, KD * VOCAB], wdt)
            w3 = w_sb.rearrange("p (k v) -> p k v", k=KD)
            enc_t = persist.tile([128, KE * TT], wdt)
            pred_t = persist.tile([128, KE * U], wdt)

            def _prologue():
                # small operands lead on the scalar ring; onehot/identr on
                # sync. W^T chunk-0 slab split across both HWDGE rings so
                # chunk 0's projections can start ASAP; chunks 1-3 slabs
                # prefetch on the gpsimd queue, overlapping chunk 0.
                nc.scalar.dma_start(out=enc_t, in_=enc_t_d[:])
                nc.scalar.dma_start(out=pred_t, in_=pred_t_d[:])
                nc.scalar.dma_start(out=consts, in_=const_d[:])
                nc.sync.dma_start(out=identr, in_=identr_d[:])
                nc.sync.dma_start(out=onehot, in_=onehot_d[:])
                nc.scalar.dma_start(out=w3[:, 0:KE, 0:VC],
                                    in_=wt_d[:, 0:KE, 0:VC])
                nc.sync.dma_start(out=w3[:, KE:KD, 0:VC],
                                  in_=wt_d[:, KE:KD, 0:VC])
                eng = nc.gpsimd if first_gp else nc.sync
                for c in range(1, NC):
                    eng.dma_start(out=w3[:, :, c * VC:(c + 1) * VC],
                                  in_=wt_d[:, :, c * VC:(c + 1) * VC])
                # PE pre-consumes the onehot/identr DMAs once so no later
                # instruction needs >2 sync-wait commands (ISA limit)
                ps_dummy = ps_s.tile([128, VC], f32, tag="ps")
                nc.tensor.matmul(ps_dummy[:, :128], lhsT=identr, rhs=identr,
                                 start=True, stop=True)
                nc.tensor.matmul(ps_dummy, lhsT=onehot[:, 0:128],
                                 rhs=onehot[:, 0:VC], start=True, stop=True)

            def _chunk(c, first=False):
                if first:
                    _prologue()
                # -- enc/pred projection chunks; the two accumulation chains
                # interleave across two PSUM banks so each matmul's drain
                # overlaps the other chain's fill
                ps = ps_s.tile([128, VC], f32, tag="ps")
                ps2 = ps_s.tile([128, VC], f32, tag="ps")
                for k in range(KE):
                    nc.tensor.matmul(
                        ps, lhsT=enc_t[:, k * TT:(k + 1) * TT],
                        rhs=w3[:, k, c * VC:(c + 1) * VC],
                        start=(k == 0), stop=(k == KE - 1))
                    nc.tensor.matmul(
                        ps2[:U], lhsT=pred_t[:, k * U:(k + 1) * U],
                        rhs=w3[:, KE + k, c * VC:(c + 1) * VC],
                        start=(k == 0), stop=False)
                if enc_wide:
                    # enc duplicated side by side: the DVE drain then adds
                    # with unit strides instead of a stride-0 broadcast AP
                    enc2 = projc.tile([128, 2 * VC], bf16, tag="enc_c")
                    enc_c = enc2[:, 0:VC]
                    nc.scalar.copy(out=enc_c, in_=ps)
                    nc.scalar.copy(out=enc2[:, VC:2 * VC], in_=ps)
                else:
                    enc2 = None
                    enc_c = projc.tile([128, VC], bf16, tag="enc_c")
                    nc.scalar.copy(out=enc_c, in_=ps)
                nc.tensor.matmul(
                    ps2[:U], lhsT=ones.broadcast_to((1, U)),
                    rhs=bias_sb[:, c * VC:(c + 1) * VC],
                    start=False, stop=True)
                pred_c = projc.tile([U, VC], bf16, tag="pred_c")
                nc.scalar.copy(out=pred_c, in_=ps2[:U])

                # -- main: stores of (128t, GU u, 512v) bf16. Each PSUM
                # tile covers 2 u's; every act_mod-th tile takes the
                # ScalarE path (PE also accumulates enc via identity-
                # matmul, ScalarE copies PSUM->SBUF), the rest the DVE
                # path (tensor_add with free-dim-broadcast enc operand).
                for g in range(n_groups):
                    o = outp.tile([128, GU * VC], odt, tag="o")
                    if mode == "store_only":
                        nc.vector.tensor_copy(o[0:1, 0:1], ones)
                    for h in range(GU // 2):
                        if mode == "store_only":
                            break
                        use_act = (h % act_mod == 0)
                        ps = ps_m.tile([128, 2 * VC], f32, tag="ps")
                        for s in range(2):
                            u = g * GU + h * 2 + s
                            nc.tensor.matmul(
                                ps[:, s * VC:(s + 1) * VC],
                                lhsT=onehot[:, u * 128:(u + 1) * 128],
                                rhs=pred_c, start=True, stop=not use_act)
                        if use_act:
                            for s in range(2):
                                nc.tensor.matmul(
                                    ps[:, s * VC:(s + 1) * VC],
                                    lhsT=identr, rhs=enc_c,
                                    start=False, stop=True)
                        if mode == "pe_only":
                            continue
                        dst = o[:, h * 2 * VC:(h + 1) * 2 * VC]
                        if use_act:
                            nc.scalar.copy(out=dst, in_=ps)
                        elif enc_wide:
                            nc.vector.tensor_add(dst, enc2, ps)
                        else:
                            nc.vector.tensor_add(
                                dst.rearrange("p (a v) -> p a v", a=2),
                                enc_c[:, None, :].broadcast_to((128, 2, VC)),
                                ps.rearrange("p (a v) -> p a v", a=2))
                    if mode in ("pe_only", "nostore"):
                        continue
                    gi = c * n_groups + g
                    if store_mode == "alt":
                        seng = (nc.sync, nc.scalar)[gi % 2]
                    elif store_mode == "rot3":
                        seng = (nc.sync, nc.scalar, nc.gpsimd)[gi % 3]
                    elif store_mode == "rot2g":
                        seng = (nc.sync, nc.gpsimd)[gi % 2]
                    else:
                        seng = nc.sync
                    # store_split>1: issue the group's store in slices as
                    # soon as each slice's drains land, hiding the DMA
                    # trigger latency behind the remaining drains
                    o3 = o.rearrange("p (a v) -> p a v", a=GU)
                    su = GU // store_split
                    for si in range(store_split):
                        seng.dma_start(
                            out=out_d[:, g * GU + si * su:
                                      g * GU + (si + 1) * su,
                                      c * VC:(c + 1) * VC],
                            in_=o3[:, si * su:(si + 1) * su, :])

            def _vblock(vb, first=False):
                # v-partitioned: per 128-v block, tiny projections with v
                # as the output partition (lhsT = the same W^T k-tiles),
                # then the (t,u) broadcast-add runs entirely on DVE+GPSIMD
                # with free-dim stride-0 APs -- no PE broadcast, no PSUM
                # drain in the main loop.
                if first:
                    _prologue()
                vs = vb * 128
                pe = ps_s.tile([128, TT], f32, tag="pse")
                pp = ps_s.tile([128, U], f32, tag="psp")
                for k in range(KE):
                    nc.tensor.matmul(
                        pe, lhsT=w3[:, k, vs:vs + 128],
                        rhs=enc_t[:, k * TT:(k + 1) * TT],
                        start=(k == 0), stop=(k == KE - 1))
                    nc.tensor.matmul(
                        pp, lhsT=w3[:, KE + k, vs:vs + 128],
                        rhs=pred_t[:, k * U:(k + 1) * U],
                        start=(k == 0), stop=False)
                # bias: K=1 matmul, lhsT = bias row slice (partition 0),
                # rhs = onehot row 0 cols 0..U (exact ones)
                nc.tensor.matmul(
                    pp, lhsT=bias_sb[:, vs:vs + 128],
                    rhs=onehot[0:1, 0:U], start=False, stop=True)
                enc_T = projc.tile([128, TT], bf16, tag="enc_T")
                nc.scalar.copy(out=enc_T, in_=pe)
                pred_T = projc.tile([128, U], bf16, tag="pred_T")
                nc.scalar.copy(out=pred_T, in_=pp)
                o = outp.tile([128, TT * U], odt, tag="o")
                o3 = o.rearrange("p (t u) -> p t u", t=TT)
                half = TT // 2
                nc.vector.tensor_add(
                    o3[:, 0:half, :],
                    enc_T[:, 0:half, None].broadcast_to((128, half, U)),
                    pred_T[:, None, :].broadcast_to((128, half, U)))
                nc.gpsimd.tensor_add(
                    o3[:, half:TT, :],
                    enc_T[:, half:TT, None].broadcast_to((128, half, U)),
                    pred_T[:, None, :].broadcast_to((128, half, U)))
                nc.sync.dma_start(out=out_d[vs:vs + 128, :, :], in_=o3)

            for _rep in range(reps):
                if mode == "vpart":
                    for vb in range(VOCAB // 128):
                        _vblock(vb, first=(_rep == 0 and vb == 0))
                else:
                    for c in range(NC):
                        _chunk(c, first=(_rep == 0 and c == 0))
    nc.compile()
    return nc


def _i8_scale(enc_out, pred_out, W, b):
    """Exact global max of |logits|: max/min over (t,u) separate, so
    max_{t,u}(e+p) = max_t e + max_u p exactly, per (batch, v)."""
    enc = np.asarray(enc_out, dtype=np.float32)
    pred = np.asarray(pred_out, dtype=np.float32)
    W = np.asarray(W, dtype=np.float32)
    b = np.asarray(b, dtype=np.float32)
    d = enc.shape[-1]
    e = enc.reshape(-1, d) @ W[:, :d].T          # (B*T, V)
    p = pred.reshape(-1, d) @ W[:, d:].T         # (B*U, V)
    e = e.reshape(B, T, VOCAB)
    p = p.reshape(B, U, VOCAB)
    hi = (e.max(1) + p.max(1) + b).max()
    lo = (e.min(1) + p.min(1) + b).min()
    return float(max(hi, -lo)) / 126.0


def _make_in_maps(enc_out, pred_out, W, b, reps=1, gu=16, act_mod=2,
                  store_mode="sync", w_f32r=True, first_gp=False,
                  mode="full", out_i8=False, enc_wide=False, obufs=3,
                  store_split=1):
    import ml_dtypes
    bf16 = ml_dtypes.bfloat16
    wnp = np.float32 if w_f32r else bf16
    W = np.asarray(W, dtype=np.float32)
    if out_i8:
        s = _i8_scale(enc_out, pred_out, W, b)
        W = W / s
        b = np.asarray(b, dtype=np.float32) / s
    # W^T in k-tile-major (128 d-partition, KD k, V) layout
    wt = np.ascontiguousarray(
        W.T.reshape(KD, 128, VOCAB).transpose(1, 0, 2)).astype(wnp)
    consts = np.zeros((1, VOCAB + 1), dtype=np.float32)
    consts[0, :VOCAB] = np.asarray(b, dtype=np.float32)
    consts[0, VOCAB] = 1.0
    consts = consts.astype(bf16)
    ident = np.eye(128, dtype=np.float32).astype(bf16)
    onehot_m = np.zeros((U, U * 128), dtype=np.float32)
    for u in range(U):
        onehot_m[u, u * 128:(u + 1) * 128] = 1.0
    onehot_m = onehot_m.astype(bf16)
    cfg = np.zeros((1 + BUILD_ID * 64 + gu + 2 * act_mod
                    + 512 * ["sync", "alt", "rot3", "rot2g"].index(store_mode)
                    + 2048 * int(w_f32r)
                    + 4096 * int(first_gp)
                    + 8192 * ["full", "pe_only", "nostore",
                              "store_only", "vpart"].index(mode)
                    + 32768 * int(out_i8)
                    + 65536 * int(enc_wide)
                    + 262144 * (obufs - 3)
                    + 524288 * (store_split - 1), reps),
                   dtype=np.float32)
    in_maps = []
    for i in range(N_CORES):
        bi, th = i // 2, i % 2
        enc = np.asarray(enc_out[bi, th * TT:(th + 1) * TT, :],
                         dtype=np.float32)  # (TT, D_ENC)
        pred = np.asarray(pred_out[bi], dtype=np.float32)  # (U, D_PRED)
        enc_t = np.ascontiguousarray(
            enc.T.reshape(KE, 128, TT).transpose(1, 0, 2)).astype(wnp)
        pred_t = np.ascontiguousarray(
            pred.T.reshape(KE, 128, U).transpose(1, 0, 2)).astype(wnp)
        in_maps.append({
            "cfg": cfg,
            "wt": wt,
            "enc_t": enc_t,
            "pred_t": pred_t,
            "consts": consts,
            "identr": ident,
            "onehotr": onehot_m,
        })
    return in_maps


OUT_I8 = False


def kernel(enc_out, pred_out, W, b):
    import os

    from concourse.bass_utils import run_bass_kernel_spmd

    if "nc" not in _cache:
        _cache["nc"] = _build(out_i8=OUT_I8)
    nc = _cache["nc"]
    trace = bool(os.environ.get("KJN_TRACE"))

    in_maps = _make_in_maps(enc_out, pred_out, W, b, out_i8=OUT_I8)
    s = _i8_scale(enc_out, pred_out, W, b) if OUT_I8 else 1.0

    kw = {}
    if trace:
        kw = dict(trace=True, trace_cores=[0], stitch_traces=False)
    res = run_bass_kernel_spmd(nc, in_maps, core_ids=list(range(N_CORES)), **kw)
    if trace:
        print(f"HW exec time: {res.exec_time_ns} ns")
        print(f"trace: {res.instructions_and_trace[1] if res.instructions_and_trace else None}")
        print(f"profile_json: {res.profile_json}")
    out = np.empty((B, T, U, VOCAB), dtype=np.float32)
    for i in range(N_CORES):
        bi, th = i // 2, i % 2
        slab = np.asarray(res.results[i]["out"], dtype=np.float32)
        if OUT_I8:
            slab *= s
        out[bi, th * TT:(th + 1) * TT] = slab
    return out
